# revision 42
# baseline (speedup 1.0000x reference)
"""Trainium2 Bass kernel for nn_Attention_38233798869191.

Full multi-head attention layer (B=2, S=2048, D=1024, H=16, dh=64) with the
reference's "faithful to original" reshape quirk, sharded over 8 NeuronCores
by splitting heads (tensor parallel): core c owns heads {2c, 2c+1}.

Per-core dataflow (everything transposed: feature dim on SBUF partitions):
  xT [1024, 4096]     (host-pretransposed x, shared by all cores)
  qT = (WqT_c.T @ xT) * SCALE   [128, 4096]   (2 heads x 64 dims)
  kT, vT likewise.
  v natural [sk, dh] built from vT via TensorE transposes, with an extra
  ones-column so the p@v matmul also produces the softmax denominators.
  Per (b, head): scoresT[sk, sq] = kT.T @ qT; p = exp(scoresT) (no max
  subtraction -- scores are O(1) by construction); oT' = [v | 1].T @ p
  accumulated over sk chunks in PSUM; transpose back, normalize by the
  denominator row, write o natural [2048, 64] to a DRAM scratch.
  The reference's o.reshape(B, D, S).swapaxes trick means the output
  projection is y[b].T = Wo @ M[b] where M[b][h*64:(h+1)*64] is simply
  o_natural[b,h] reinterpreted as [64, 2048] (contiguous reshape), so the
  scratch is DMA'd back as [64, 2048] rows of M. Each core computes the
  partial y[b].T = Wo[:, c*128:(c+1)*128] @ M_c[b]; partials are summed on
  the host (the tensor-parallel all-reduce) and transposed back.
"""

import os
import sys

import numpy as np

for _p in ("/opt/trn_rl_repo", "/root/.axon_site/_ro/trn_rl_repo"):
    if os.path.isdir(_p) and _p not in sys.path:
        sys.path.insert(0, _p)

B, S, D, H, DH = 2, 2048, 1024, 16, 64
NSEQ = B * S  # 4096
SCALE = 1.0 / float(np.sqrt(DH))
N_CORES = 8
P = 128

# Schraudolph fast-exp constants: exp(x) ~= bitcast_f32(int32(x*A + B)).
# C calibrated for min max-rel-err (~2.98%) over x in [-3, 3] with
# round-to-nearest f32->i32 conversion (verified against CoreSim DVE).
SCHR_A = float(2**23 / np.log(2.0))
SCHR_B = float(127 * 2**23 - 366500)
# 16-bit variant: exp(x) ~= bitcast_bf16(int16(x*A16 + B16)), ~3.3% max rel
# err. Used because the fp32r BIR verifier rejects bitcast producers; the
# p@v matmul runs in bf16 instead (same PE rate).
SCHR_A16 = float(2**7 / np.log(2.0))
SCHR_B16 = float(127 * 128 - 5.5)

# "f32r": fp32 data, matmuls in float32r (full-rate fp32 mode)
# "bf16": bf16 data + matmuls
# "f32": fp32 data, plain fp32 matmuls (4x slower, reference mode)
DTYPE_MODE = os.environ.get("KERNEL_DTYPE_MODE", "f32r")
# bisect/tuning knobs
NO_SCHR = os.environ.get("KERNEL_NO_SCHR", "0") == "1"  # exp all on ACT
PV_F32R = os.environ.get("KERNEL_PV_F32R", "0") == "1"  # p@v in f32r


def _build_nc(mode, reps=1):
    import concourse.bass as bass  # noqa: F401
    import concourse.mybir as mybir
    import concourse.tile as tile
    from concourse import bacc
    from concourse.masks import make_identity

    f32 = mybir.dt.float32
    if mode == "bf16":
        elt = mybir.dt.bfloat16
        mmdt = mybir.dt.bfloat16
    elif mode == "f32r":
        # float32r tiles everywhere: the BIR verifier requires every producer
        # feeding an fp32r matmul to emit fp32r-rounded values.
        elt = mybir.dt.float32r
        mmdt = mybir.dt.float32r
    else:
        elt = f32
        mmdt = f32
    AF = mybir.ActivationFunctionType

    nc = bacc.Bacc(
        "TRN2",
        target_bir_lowering=False,
        debug=False,
        num_devices=N_CORES,
    )

    xT = nc.dram_tensor("xT", [D, NSEQ], elt, kind="ExternalInput")
    wqT = nc.dram_tensor("wqT", [D, P], elt, kind="ExternalInput")
    wkT = nc.dram_tensor("wkT", [D, P], elt, kind="ExternalInput")
    wvT = nc.dram_tensor("wvT", [D, P], elt, kind="ExternalInput")
    woT = nc.dram_tensor("woT", [P, D], elt, kind="ExternalInput")
    bqs = nc.dram_tensor("bqs", [P, 1], f32, kind="ExternalInput")  # bq*SCALE
    bk = nc.dram_tensor("bk", [P, 1], f32, kind="ExternalInput")
    bv = nc.dram_tensor("bv", [P, 1], f32, kind="ExternalInput")
    bf16 = mybir.dt.bfloat16
    i32 = mybir.dt.int32
    i16 = mybir.dt.int16
    # the p@v side runs in bf16 (p in [0, e^3], v well-scaled; psum
    # accumulates f32) so the DVE Schraudolph halves can feed it directly
    pv_dt = elt if (mode == "f32" or PV_F32R) else bf16
    # partial y written bf16: halves the dominant DMA transfer (the host
    # all-reduce sums 8 partials, so bf16 partial noise stays ~3e-4 rel)
    ypT = nc.dram_tensor("ypT", [B, D, S], bf16, kind="ExternalOutput")
    osc = nc.dram_tensor("osc", [2 * 2, S, DH], elt)  # o natural per (b, hl)

    # DRAM views
    # k global = ko*512 + ks*128 + p
    xTv = xT.ap().rearrange("(ko ks p) s -> ko p ks s", ks=4, p=P)
    # flat view: M row r of head h == osc[h] elements [r*2048, (r+1)*2048)
    # (contiguous, so the M readback DMA gets full 8KB descriptors)
    oscF = osc.ap().rearrange("h s d -> h (s d)")  # [4, 131072]

    def wview(w):
        return w.ap().rearrange("(kc p) m -> p kc m", p=P)  # [128, 8, 128]

    with tile.TileContext(nc) as tc:
        with tc.tile_pool(name="persist", bufs=1) as pp:
            # persistent SBUF tensors
            # stage weight loads: the very first proj matmuls only need the
            # ko=0 quarter of wq plus the first x chunk; everything else
            # queues behind them (emitted via the proj_batch hook below)
            w_sb = {}
            wv_views = {}
            for name, w in (("q", wqT), ("k", wkT), ("v", wvT)):
                w_sb[name] = pp.tile([P, 8, P], elt, tag=f"w{name}", name=f"w{name}")
                wv_views[name] = wview(w)
            nc.sync.dma_start(w_sb["q"][:, 0:4, :], wv_views["q"][:, 0:4, :])
            bias_sb = {}
            for name, bt in (("q", bqs), ("k", bk), ("v", bv)):
                bias_sb[name] = pp.tile([P, 1], f32, tag=f"b{name}", name=f"b{name}")

            def load_rest_of_weights():
                for name in ("k", "v"):
                    nc.sync.dma_start(
                        w_sb[name][:, 0:4, :], wv_views[name][:, 0:4, :]
                    )
                for name, bt in (("q", bqs), ("k", bk), ("v", bv)):
                    nc.sync.dma_start(bias_sb[name][:], bt.ap())
                for name in ("q", "k", "v"):
                    nc.sync.dma_start(
                        w_sb[name][:, 4:8, :], wv_views[name][:, 4:8, :]
                    )

            woT_sb = pp.tile([P, D], elt, tag="wo", name="wo")
            # vT only feeds TensorE transposes (not fp32r matmuls), so in
            # f32r mode it stays plain f32 (memset/make_identity/transpose
            # all dislike f32r); rounding to f32r happens at the v_nat copy.
            vt_dt = f32 if mode == "f32r" else elt
            qT_sb = pp.tile([P, NSEQ], elt, tag="qT", name="qT")
            kT_sb = pp.tile([P, NSEQ], elt, tag="kT", name="kT")
            vT_sb = pp.tile([P, NSEQ], vt_dt, tag="vT", name="vT")
            proj_sb = {"q": qT_sb, "k": kT_sb, "v": vT_sb}
            M_sb = [pp.tile([P, S], elt, tag=f"M{b}", name=f"M{b}") for b in range(B)]
            v_nat = [pp.tile([P, 16, 72], pv_dt, tag=f"vn{i}", name=f"vn{i}") for i in range(4)]
            # stacked identity: I64 in both partition halves, so transposes of
            # operands based at partition 0 or 64 both have a matching rhs
            id2 = pp.tile([P, 64], vt_dt, tag="id2", name="id2")
            make_identity(nc, id2[0:64, :])
            make_identity(nc, id2[64:128, :])
            id_f32 = pp.tile([P, P], f32, tag="id_f32", name="id_f32")
            make_identity(nc, id_f32[:])
            ones_sb = pp.tile([P, 16], f32, tag="ones", name="ones")
            nc.vector.memset(ones_sb[:], 1.0)

            for _rep in range(reps):
                with (
                    tc.tile_pool(name="xin", bufs=6) as xpool,
                    # one shared PSUM budget (8 banks):
                    #   psp: 6 x [128,512] f32 = 6 banks -- shared ring for
                    #        proj accums, score halves, outproj halves, and
                    #        all TensorE transposes
                    #   ops: 1 x [65,1024] f32 = 2 banks (o accumulator)
                    tc.tile_pool(name="psp", bufs=6, space="PSUM") as psp,
                    tc.tile_pool(name="ops", bufs=1, space="PSUM") as ops,
                    tc.tile_pool(name="ptp", bufs=14) as ptp,
                    tc.tile_pool(name="otp", bufs=2) as otp,
                    tc.tile_pool(name="obp", bufs=2) as obp,
                    tc.tile_pool(name="rcp", bufs=3) as rcp,
                    tc.tile_pool(name="ysb", bufs=4) as ysbp,
                ):

                    def ptile_psum(cols=512):
                        """[128, cols] f32 slot from the shared psum ring."""
                        return psp.tile([P, 512], f32, tag="ps", name="ps")

                    def proj_batch(bi, after_first_x=None):
                        """q/k/v projections for batch bi's sequence columns
                        (+ the ones-columns of its v_nat tiles)."""
                        for sq in range(bi * 4, bi * 4 + 4):
                            acc = {}
                            for n in "qkv":
                                acc[n] = ptile_psum()
                            for ko in range(2):
                                x_sb = xpool.tile(
                                    [P, 4, 512], elt, tag="x", name="x"
                                )
                                for xh in range(2):
                                    nc.sync.dma_start(
                                        x_sb[:, xh * 2 : (xh + 1) * 2, :],
                                        xTv[
                                            ko,
                                            :,
                                            xh * 2 : (xh + 1) * 2,
                                            sq * 512 : (sq + 1) * 512,
                                        ],
                                    )
                                if after_first_x is not None:
                                    after_first_x()
                                    after_first_x = None
                                for n in "qkv":
                                    for ks in range(4):
                                        nc.tensor.matmul(
                                            acc[n][:],
                                            w_sb[n][
                                                :, ko * 4 + ks, :
                                            ].bitcast(mmdt),
                                            x_sb[:, ks, :].bitcast(mmdt),
                                            start=(ko == 0 and ks == 0),
                                            stop=(ko == 1 and ks == 3),
                                        )
                            sl = slice(sq * 512, (sq + 1) * 512)
                            nc.scalar.activation(
                                qT_sb[:, sl], acc["q"][:], AF.Identity,
                                bias=bias_sb["q"][:], scale=SCALE,
                            )
                            nc.scalar.activation(
                                kT_sb[:, sl], acc["k"][:], AF.Identity,
                                bias=bias_sb["k"][:],
                            )
                            nc.scalar.activation(
                                vT_sb[:, sl], acc["v"][:], AF.Identity,
                                bias=bias_sb["v"][:],
                            )
                        for hl in range(2):
                            nc.vector.tensor_copy(
                                v_nat[bi * 2 + hl][:, :, 64:65],
                                ones_sb[:, :, None],
                            )

                    def vnat_units(bi, hl):
                        """16 closures: transpose vT columns into v natural
                        [sk, dh] for head (bi, hl). Interleaved into an
                        attention loop so the psum-ring waits hide."""
                        bh = bi * 2 + hl
                        hsl = slice(hl * 64, (hl + 1) * 64)

                        def unit(t):
                            c0 = bi * S + t * P
                            ps = ptile_psum()
                            if vt_dt is f32:
                                pt = ps[:, 0:64]
                            else:
                                pt = ps[:].bitcast(vt_dt)[:, 0:64]
                            nc.tensor.transpose(
                                pt, vT_sb[hsl, c0 : c0 + P], id2[hsl, :]
                            )
                            if t % 2 == 0:
                                nc.vector.tensor_copy(
                                    v_nat[bh][:, t, 0:64], pt
                                )
                            else:
                                nc.scalar.copy(v_nat[bh][:, t, 0:64], pt)

                        return [(lambda t=t: unit(t)) for t in range(16)]

                    _ob_live = {}

                    def norm_units(bh, sqh, ot):
                        """8 closures: transpose back + divide by the
                        denominator row + store o into M (via DRAM scratch,
                        or an SBUF scramble for the last head)."""
                        b, hl = bh // 2, bh % 2

                        def unit(tb):
                            if tb == 0:
                                _ob_live[(bh, sqh)] = obp.tile(
                                    [P, 8, DH], elt, tag="ob", name="ob"
                                )
                            ob = _ob_live[(bh, sqh)]
                            ps = ptile_psum()
                            pt2 = ps[:, 0:72]
                            nc.tensor.transpose(
                                pt2[:, :65],
                                ot[:, tb * P : (tb + 1) * P],
                                id_f32[:65, :65],
                            )
                            rc = rcp.tile([P, 1], f32, tag="rc", name="rc")
                            nc.vector.reciprocal(rc[:], pt2[:, 64:65])
                            if tb % 2 == 0:
                                nc.scalar.activation(
                                    ob[:, tb, :], pt2[:, 0:64], AF.Copy,
                                    scale=rc[:],
                                )
                            else:
                                nc.vector.tensor_scalar_mul(
                                    ob[:, tb, :], pt2[:, 0:64], rc[:]
                                )
                            if tb == 7:
                                s0 = sqh * 1024
                                nc.sync.dma_start(
                                    osc.ap()[
                                        bh, s0 : s0 + 1024, :
                                    ].rearrange("(t p) d -> p t d", p=P),
                                    ob[:],
                                )
                                r0 = sqh * 32
                                nc.sync.dma_start(
                                    M_sb[b][
                                        hl * 64 + r0 : hl * 64 + r0 + 32, :
                                    ],
                                    oscF[
                                        bh, r0 * 2048 : (r0 + 32) * 2048
                                    ].rearrange("(r c) -> r c", c=2048),
                                )

                        return [(lambda tb=tb: unit(tb)) for tb in range(8)]

                    def attention_head(b, hl, extra=None, norm_self=False):
                        """Returns the two ot (o-transposed+denominator)
                        SBUF tiles; caller schedules their normalization.
                        `extra`: closures drained one per half-iteration
                        (64 slots). `norm_self`: normalize in-head (last
                        head)."""
                        bh = b * 2 + hl
                        hsl = slice(hl * 64, (hl + 1) * 64)
                        extra = extra if extra is not None else []
                        ots = []
                        for sqh in range(2):  # halves of 1024 queries
                            sq0 = b * S + sqh * 1024
                            po = ops.tile(
                                [65, 1024], f32, tag="oacc", name="oacc"
                            )

                            def emit_pv(kc, half, pt):
                                nc.tensor.matmul(
                                    po[:, half * 512 : (half + 1) * 512],
                                    v_nat[bh][:, kc, 0:65],
                                    pt[:],
                                    start=(kc == 0),
                                    stop=(kc == 15),
                                )

                            # software-pipelined: pv lags ~2 kc behind qk;
                            # exp alternates ACT (table exp) and DVE
                            # (Schraudolph int-trick, ~3% max rel err; the
                            # denominator uses the same approx values so
                            # common-mode cancels) per 512-query half-tile
                            pending_pv = []
                            for kc in range(16):
                                k0 = b * S + kc * P
                                for half in range(2):
                                    ps = ptile_psum()
                                    nc.tensor.matmul(
                                        ps[:],
                                        kT_sb[hsl, k0 : k0 + P].bitcast(mmdt),
                                        qT_sb[
                                            hsl,
                                            sq0 + half * 512 : sq0
                                            + (half + 1) * 512,
                                        ].bitcast(mmdt),
                                        start=True,
                                        stop=True,
                                    )
                                    pt = ptp.tile(
                                        [P, 512], pv_dt, tag="pt", name="pt"
                                    )
                                    if (mode != "f32" and not NO_SCHR
                                            and (kc + half) % 2):
                                        nc.vector.tensor_scalar(
                                            pt[:].bitcast(i16), ps[:],
                                            SCHR_A16, SCHR_B16,
                                            mybir.AluOpType.mult,
                                            mybir.AluOpType.add,
                                        )
                                    else:
                                        nc.scalar.activation(
                                            pt[:], ps[:], AF.Exp
                                        )
                                    pending_pv.append((kc, half, pt))
                                    # deep lag (~4 kc): by the time a pv
                                    # reaches the PE queue head its exp sem
                                    # has long fired, so the in-order queue
                                    # never stalls and LDWEIGHTS prefetches
                                    if len(pending_pv) > 8:
                                        emit_pv(*pending_pv.pop(0))
                                    if extra:
                                        extra.pop(0)()
                            for args in pending_pv:
                                emit_pv(*args)

                            ot = otp.tile([65, 1024], f32, tag="ot", name="ot")
                            # split po evict across both engines: frees the
                            # 2-bank po accumulator sooner for the next half
                            nc.scalar.copy(ot[:, 0:512], po[:, 0:512])
                            nc.vector.tensor_copy(
                                ot[:, 512:1024], po[:, 512:1024]
                            )
                            ots.append(ot)
                            if norm_self and sqh == 0:
                                # queue own first-half normalize into the
                                # second half's slots
                                extra.extend(norm_units(bh, 0, ot))
                        if norm_self:
                            for u in extra:  # leftovers
                                u()
                            for u in norm_units(bh, 1, ots[1]):
                                u()
                        return ots

                    _ysb_live = {}

                    def outproj_halves(b):
                        """32 closures: one 512-col matmul + one psum-evict
                        each; evict engine alternates ACT/DVE."""

                        def unit(mo, nh, half):
                            if nh == 0 and half == 0:
                                _ysb_live[(b, mo)] = ysbp.tile(
                                    [P, 2 * 1024], bf16, tag="y", name="y"
                                )
                            ysb = _ysb_live[(b, mo)]
                            py = ptile_psum()
                            n0 = nh * 1024 + half * 512
                            nc.tensor.matmul(
                                py[:],
                                woT_sb[:, mo * P : (mo + 1) * P].bitcast(mmdt),
                                M_sb[b][:, n0 : n0 + 512].bitcast(mmdt),
                                start=True,
                                stop=True,
                            )
                            if (mo + nh + half) % 2 == 0:
                                nc.vector.tensor_copy(
                                    ysb[:, n0 : n0 + 512], py[:]
                                )
                            else:
                                nc.scalar.copy(ysb[:, n0 : n0 + 512], py[:])
                            if nh == 1 and half == 1:
                                nc.sync.dma_start(
                                    ypT.ap()[b, mo * P : (mo + 1) * P, :],
                                    ysb[:],
                                )

                        return [
                            (lambda mo=mo, nh=nh, half=half: unit(mo, nh, half))
                            for mo in range(8)
                            for nh in range(2)
                            for half in range(2)
                        ]

                    proj_batch(
                        0,
                        after_first_x=(
                            load_rest_of_weights if _rep == 0 else None
                        ),
                    )
                    if _rep == 0:
                        nc.sync.dma_start(woT_sb[:], woT.ap())
                    ot00 = attention_head(
                        0, 0, extra=vnat_units(0, 0) + vnat_units(0, 1)
                    )
                    proj_batch(1)
                    ot01 = attention_head(
                        0, 1,
                        extra=norm_units(0, 0, ot00[0])
                        + norm_units(0, 1, ot00[1])
                        + vnat_units(1, 0),
                    )
                    ot10 = attention_head(
                        1, 0,
                        extra=norm_units(1, 0, ot01[0])
                        + norm_units(1, 1, ot01[1])
                        + vnat_units(1, 1),
                    )
                    op0 = outproj_halves(b=0)
                    attention_head(
                        1, 1,
                        extra=norm_units(2, 0, ot10[0])
                        + norm_units(2, 1, ot10[1])
                        + op0[:16],
                        norm_self=True,
                    )
                    # outproj(0) leftovers fill the window while the last
                    # head's scramble DMAs land in M (emitted after
                    # norm(3,1) so those DMAs are first in the queue);
                    # then the exposed outproj(1)
                    for u in op0[16:]:
                        u()
                    for u in outproj_halves(b=1):
                        u()

    nc.compile()
    return nc


_CACHE = {}


def _np_elt(mode):
    if mode == "bf16":
        import ml_dtypes

        return ml_dtypes.bfloat16
    return np.float32


def _get_runner(mode, reps=1):
    """Build (once) the compiled kernel + a persistent jitted executor."""
    key = (mode, reps)
    if key in _CACHE:
        return _CACHE[key]

    import jax
    import jax.numpy as jnp  # noqa: F401
    from jax.sharding import Mesh, PartitionSpec
    from jax.experimental.shard_map import shard_map
    import concourse.mybir as mybir
    from concourse import bass2jax

    nc = _build_nc(mode, reps)
    bass2jax.install_neuronx_cc_hook()

    partition_name = (
        nc.partition_id_tensor.name if nc.partition_id_tensor else None
    )
    in_names = []
    out_names = []
    out_avals = []
    for alloc in nc.m.functions[0].allocations:
        if not isinstance(alloc, mybir.MemoryLocationSet):
            continue
        name = alloc.memorylocations[0].name
        if alloc.kind == "ExternalInput":
            if name != partition_name:
                in_names.append(name)
        elif alloc.kind == "ExternalOutput":
            out_names.append(name)
            shape = tuple(alloc.tensor_shape)
            dtype = mybir.dt.np(alloc.dtype)
            out_avals.append(jax.core.ShapedArray(shape, dtype))
    n_params = len(in_names)
    n_outs = len(out_avals)
    all_in_names = list(in_names) + list(out_names)
    if partition_name is not None:
        all_in_names.append(partition_name)
    all_in_names = tuple(all_in_names)

    def _body(*args):
        operands = list(args)
        if partition_name is not None:
            operands.append(bass2jax.partition_id_tensor())
        outs = bass2jax._bass_exec_p.bind(
            *operands,
            out_avals=tuple(out_avals),
            in_names=all_in_names,
            out_names=tuple(out_names),
            lowering_input_output_aliases=(),
            sim_require_finite=True,
            sim_require_nnan=True,
            nc=nc,
        )
        return tuple(outs)

    devices = jax.devices()[:N_CORES]
    mesh = Mesh(np.asarray(devices), ("core",))
    in_specs = (PartitionSpec("core"),) * (n_params + n_outs)
    out_specs = (PartitionSpec("core"),) * n_outs
    donate = tuple(range(n_params, n_params + n_outs))
    sharded = jax.jit(
        shard_map(
            _body, mesh=mesh, in_specs=in_specs, out_specs=out_specs,
            check_rep=False,
        ),
        donate_argnums=donate,
        keep_unused=True,
    )

    zero_out_shapes = [
        ((N_CORES * a.shape[0],) + tuple(a.shape[1:]), a.dtype)
        for a in out_avals
    ]

    def execute(in_maps):
        concat_in = [
            np.concatenate([np.asarray(m[name]) for m in in_maps], axis=0)
            for name in in_names
        ]
        concat_zeros = [np.zeros(s, d) for s, d in zero_out_shapes]
        out_arrs = sharded(*concat_in, *concat_zeros)
        out_arrs = [np.asarray(o) for o in out_arrs]
        return [
            {
                name: out_arrs[i].reshape(
                    N_CORES, *out_avals[i].shape
                )[c]
                for i, name in enumerate(out_names)
            }
            for c in range(N_CORES)
        ]

    execute.nc = nc
    execute.in_names = in_names
    execute.out_names = out_names
    execute.out_avals = out_avals
    execute.n_params = n_params
    execute.body = _body
    execute.mesh = mesh
    execute.zero_out_shapes = zero_out_shapes
    _CACHE[key] = execute
    return execute


def make_in_maps(x, Wq, bq, Wk, bk, Wv, bv, Wo, bo, mode=None):
    mode = mode or DTYPE_MODE
    ne = _np_elt(mode)
    x = np.asarray(x, np.float32)
    xT = np.ascontiguousarray(x.reshape(NSEQ, D).T).astype(ne)
    in_maps = []
    for c in range(N_CORES):
        sl = slice(c * P, (c + 1) * P)
        in_maps.append(
            {
                "xT": xT,
                "wqT": np.ascontiguousarray(np.asarray(Wq)[sl, :].T).astype(ne),
                "wkT": np.ascontiguousarray(np.asarray(Wk)[sl, :].T).astype(ne),
                "wvT": np.ascontiguousarray(np.asarray(Wv)[sl, :].T).astype(ne),
                "woT": np.ascontiguousarray(np.asarray(Wo)[:, sl].T).astype(ne),
                "bqs": (np.asarray(bq, np.float32)[sl] * SCALE).reshape(P, 1),
                "bk": np.asarray(bk, np.float32)[sl].reshape(P, 1).copy(),
                "bv": np.asarray(bv, np.float32)[sl].reshape(P, 1).copy(),
            }
        )
    return in_maps


def kernel(x, Wq, bq, Wk, bk, Wv, bv, Wo, bo):
    mode = DTYPE_MODE
    execute = _get_runner(mode)
    in_maps = make_in_maps(x, Wq, bq, Wk, bk, Wv, bv, Wo, bo, mode)
    results = execute(in_maps)
    ysum = np.zeros((B, D, S), np.float64)
    for c in range(N_CORES):
        ysum += np.asarray(results[c]["ypT"], np.float32)
    y = ysum.transpose(0, 2, 1) + np.asarray(bo, np.float32)[None, None, :]
    return np.ascontiguousarray(y.astype(np.float32))



# revision 58
# speedup vs baseline: 2.6816x; 2.6816x over previous
"""Trainium2 Bass kernel for nn_Attention_38233798869191.

Full multi-head attention layer (B=2, S=2048, D=1024, H=16, dh=64) with the
reference's "faithful to original" reshape quirk, sharded over 8 NeuronCores
by splitting heads (tensor parallel): core c owns heads {2c, 2c+1}.

Per-core dataflow (everything transposed: feature dim on SBUF partitions):
  xT [1024, 4096]     (host-pretransposed x, shared by all cores)
  qT = (WqT_c.T @ xT) * SCALE   [128, 4096]   (2 heads x 64 dims)
  kT, vT likewise (projections in fp32r; q/k/v stored bf16).
  v natural [sk, dh] built from vT via TensorE transposes, with an extra
  ones-column so the p@v matmul also produces the softmax denominators.
  Per (b, head): scoresT[sk, sq] = kT.T @ qT in bf16 per 512-query
  half-tile into a shared 6-slot single-bank PSUM ring; p = exp(scoresT)
  on ACT (no max subtraction -- scores are O(1) by construction);
  oT' = [v | 1].T @ p in bf16, software-pipelined ~4 kc behind the scores
  (a deep lag keeps the in-order PE queue from stalling on exp semaphores,
  which would expose every stationary-weight load). Transpose back,
  normalize by the denominator row, write o natural to a DRAM scratch and
  read back this half's M rows immediately.
  The reference's o.reshape(B, D, S).swapaxes trick means the output
  projection is y[b].T = Wo @ M[b] where M[b][h*64:(h+1)*64] is simply
  o_natural[b,h] reinterpreted as [64, 2048] (contiguous reshape). Each
  core computes the partial y[b].T = Wo[:, c*128:(c+1)*128] @ M_c[b] in
  fp32r, evicting each PSUM half on alternating ACT/DVE; partials are
  written bf16 and summed on the host (the tensor-parallel all-reduce).

Scheduling: the v_nat builds, o-normalizations, and batch-0 output
projection are split into small closures interleaved into the following
attention head's score loop ("extra" slots), so the PE stream never
drains on cross-engine dependencies; the batch-1 output projection (which
needs the last head's M rows) is the only exposed tail.
"""

import os
import sys

import numpy as np

for _p in ("/opt/trn_rl_repo", "/root/.axon_site/_ro/trn_rl_repo"):
    if os.path.isdir(_p) and _p not in sys.path:
        sys.path.insert(0, _p)

B, S, D, H, DH = 2, 2048, 1024, 16, 64
NSEQ = B * S  # 4096
SCALE = 1.0 / float(np.sqrt(DH))
N_CORES = 8
P = 128

# Schraudolph fast-exp constants: exp(x) ~= bitcast_f32(int32(x*A + B)).
# C calibrated for min max-rel-err (~2.98%) over x in [-3, 3] with
# round-to-nearest f32->i32 conversion (verified against CoreSim DVE).
SCHR_A = float(2**23 / np.log(2.0))
SCHR_B = float(127 * 2**23 - 366500)
# 16-bit variant: exp(x) ~= bitcast_bf16(int16(x*A16 + B16)), ~3.3% max rel
# err. Used because the fp32r BIR verifier rejects bitcast producers; the
# p@v matmul runs in bf16 instead (same PE rate).
SCHR_A16 = float(2**7 / np.log(2.0))
SCHR_B16 = float(127 * 128 - 5.5)

# "f32r": fp32 data, matmuls in float32r (full-rate fp32 mode)
# "bf16": bf16 data + matmuls
# "f32": fp32 data, plain fp32 matmuls (4x slower, reference mode)
DTYPE_MODE = os.environ.get("KERNEL_DTYPE_MODE", "f32r")
# bisect/tuning knobs (env defaults; overridable per _build_nc call)
NO_SCHR = os.environ.get("KERNEL_NO_SCHR", "1") == "1"  # exp all on ACT
PV_F32R = os.environ.get("KERNEL_PV_F32R", "0") == "1"  # p@v in f32r
PV_LAG = int(os.environ.get("KERNEL_PV_LAG", "8"))  # pv pipeline depth
QK_BF16 = os.environ.get("KERNEL_QK_BF16", "1") == "1"  # q/k in bf16
V_BF16 = os.environ.get("KERNEL_V_BF16", "0") == "1"  # vT/transposes bf16


def _build_nc(mode, reps=1, no_schr=None, pv_f32r=None, pv_lag=None,
              qk_bf16=None, v_bf16=None):
    no_schr = NO_SCHR if no_schr is None else no_schr
    pv_f32r = PV_F32R if pv_f32r is None else pv_f32r
    pv_lag = PV_LAG if pv_lag is None else pv_lag
    qk_bf16 = QK_BF16 if qk_bf16 is None else qk_bf16
    v_bf16 = V_BF16 if v_bf16 is None else v_bf16
    import concourse.bass as bass  # noqa: F401
    import concourse.mybir as mybir
    import concourse.tile as tile
    from concourse import bacc
    from concourse.masks import make_identity

    f32 = mybir.dt.float32
    if mode == "bf16":
        elt = mybir.dt.bfloat16
        mmdt = mybir.dt.bfloat16
    elif mode == "f32r":
        # float32r tiles everywhere: the BIR verifier requires every producer
        # feeding an fp32r matmul to emit fp32r-rounded values.
        elt = mybir.dt.float32r
        mmdt = mybir.dt.float32r
    else:
        elt = f32
        mmdt = f32
    AF = mybir.ActivationFunctionType

    nc = bacc.Bacc(
        "TRN2",
        target_bir_lowering=False,
        debug=False,
        num_devices=N_CORES,
    )

    xT = nc.dram_tensor("xT", [D, NSEQ], elt, kind="ExternalInput")
    wqT = nc.dram_tensor("wqT", [D, P], elt, kind="ExternalInput")
    wkT = nc.dram_tensor("wkT", [D, P], elt, kind="ExternalInput")
    wvT = nc.dram_tensor("wvT", [D, P], elt, kind="ExternalInput")
    woT = nc.dram_tensor("woT", [P, D], elt, kind="ExternalInput")
    bqs = nc.dram_tensor("bqs", [P, 1], f32, kind="ExternalInput")  # bq*SCALE
    bk = nc.dram_tensor("bk", [P, 1], f32, kind="ExternalInput")
    bv = nc.dram_tensor("bv", [P, 1], f32, kind="ExternalInput")
    bf16 = mybir.dt.bfloat16
    i32 = mybir.dt.int32
    i16 = mybir.dt.int16
    # the p@v side runs in bf16 (p in [0, e^3], v well-scaled; psum
    # accumulates f32) so the DVE Schraudolph halves can feed it directly
    pv_dt = elt if (mode == "f32" or pv_f32r) else bf16
    # q/k optionally bf16: halves the 2-pass fp32r stationary loads in the
    # scores loop and keeps the attention phase in one PE dtype
    qk_dt = bf16 if (qk_bf16 and mode == "f32r") else elt
    qk_mm = qk_dt if qk_dt is bf16 else mmdt
    # partial y written bf16: halves the dominant DMA transfer (the host
    # all-reduce sums 8 partials, so bf16 partial noise stays ~3e-4 rel)
    ypT = nc.dram_tensor("ypT", [B, D, S], bf16, kind="ExternalOutput")
    osc = nc.dram_tensor("osc", [2 * 2, S, DH], elt)  # o natural per (b, hl)

    # DRAM views
    # k global = ko*512 + ks*128 + p
    xTv = xT.ap().rearrange("(ko ks p) s -> ko p ks s", ks=4, p=P)
    # flat view: M row r of head h == osc[h] elements [r*2048, (r+1)*2048)
    # (contiguous, so the M readback DMA gets full 8KB descriptors)
    oscF = osc.ap().rearrange("h s d -> h (s d)")  # [4, 131072]

    def wview(w):
        return w.ap().rearrange("(kc p) m -> p kc m", p=P)  # [128, 8, 128]

    with tile.TileContext(nc) as tc:
        with tc.tile_pool(name="persist", bufs=1) as pp:
            # persistent SBUF tensors
            # stage weight loads: the very first proj matmuls only need the
            # ko=0 quarter of wq plus the first x chunk; everything else
            # queues behind them (emitted via the proj_batch hook below)
            w_sb = {}
            wv_views = {}
            for name, w in (("q", wqT), ("k", wkT), ("v", wvT)):
                w_sb[name] = pp.tile([P, 8, P], elt, tag=f"w{name}", name=f"w{name}")
                wv_views[name] = wview(w)
            nc.sync.dma_start(w_sb["q"][:, 0:4, :], wv_views["q"][:, 0:4, :])
            bias_sb = {}
            for name, bt in (("q", bqs), ("k", bk), ("v", bv)):
                bias_sb[name] = pp.tile([P, 1], f32, tag=f"b{name}", name=f"b{name}")

            def load_rest_of_weights():
                for name in ("k", "v"):
                    nc.sync.dma_start(
                        w_sb[name][:, 0:4, :], wv_views[name][:, 0:4, :]
                    )
                for name, bt in (("q", bqs), ("k", bk), ("v", bv)):
                    nc.sync.dma_start(bias_sb[name][:], bt.ap())
                for name in ("q", "k", "v"):
                    nc.sync.dma_start(
                        w_sb[name][:, 4:8, :], wv_views[name][:, 4:8, :]
                    )

            woT_sb = pp.tile([P, D], elt, tag="wo", name="wo")
            # vT only feeds TensorE transposes (not fp32r matmuls), so in
            # f32r mode it stays plain f32 (memset/make_identity/transpose
            # all dislike f32r); rounding to f32r happens at the v_nat copy.
            # With v_bf16 the transposes run in bf16 (1 cyc/row vs 2).
            if v_bf16 and mode == "f32r":
                vt_dt = bf16
            else:
                vt_dt = f32 if mode == "f32r" else elt
            qT_sb = pp.tile([P, NSEQ], qk_dt, tag="qT", name="qT")
            kT_sb = pp.tile([P, NSEQ], qk_dt, tag="kT", name="kT")
            vT_sb = pp.tile([P, NSEQ], vt_dt, tag="vT", name="vT")
            proj_sb = {"q": qT_sb, "k": kT_sb, "v": vT_sb}
            M_sb = [pp.tile([P, S], elt, tag=f"M{b}", name=f"M{b}") for b in range(B)]
            v_nat = [pp.tile([P, 16, 72], pv_dt, tag=f"vn{i}", name=f"vn{i}") for i in range(4)]
            # stacked identity: I64 in both partition halves, so transposes of
            # operands based at partition 0 or 64 both have a matching rhs
            id2 = pp.tile([P, 64], vt_dt, tag="id2", name="id2")
            make_identity(nc, id2[0:64, :])
            make_identity(nc, id2[64:128, :])
            id_f32 = pp.tile([P, P], f32, tag="id_f32", name="id_f32")
            make_identity(nc, id_f32[:])
            ones_sb = pp.tile([P, 16], f32, tag="ones", name="ones")
            nc.vector.memset(ones_sb[:], 1.0)

            for _rep in range(reps):
                with (
                    tc.tile_pool(name="xin", bufs=6) as xpool,
                    # one shared PSUM budget (8 banks):
                    #   psp: 6 x [128,512] f32 = 6 banks -- shared ring for
                    #        proj accums, score halves, outproj halves, and
                    #        all TensorE transposes
                    #   ops: 1 x [65,1024] f32 = 2 banks (o accumulator)
                    tc.tile_pool(name="psp", bufs=6, space="PSUM") as psp,
                    tc.tile_pool(name="ops", bufs=1, space="PSUM") as ops,
                    tc.tile_pool(name="ptp", bufs=14) as ptp,
                    tc.tile_pool(name="otp", bufs=2) as otp,
                    tc.tile_pool(name="obp", bufs=2) as obp,
                    tc.tile_pool(name="rcp", bufs=3) as rcp,
                    tc.tile_pool(name="ysb", bufs=4) as ysbp,
                ):

                    def ptile_psum(cols=512):
                        """[128, cols] f32 slot from the shared psum ring."""
                        return psp.tile([P, 512], f32, tag="ps", name="ps")

                    def proj_batch(bi, after_first_x=None):
                        """q/k/v projections for batch bi's sequence columns
                        (+ the ones-columns of its v_nat tiles)."""
                        for sq in range(bi * 4, bi * 4 + 4):
                            acc = {}
                            for n in "qkv":
                                acc[n] = ptile_psum()
                            for ko in range(2):
                                x_sb = xpool.tile(
                                    [P, 4, 512], elt, tag="x", name="x"
                                )
                                for xh in range(2):
                                    nc.sync.dma_start(
                                        x_sb[:, xh * 2 : (xh + 1) * 2, :],
                                        xTv[
                                            ko,
                                            :,
                                            xh * 2 : (xh + 1) * 2,
                                            sq * 512 : (sq + 1) * 512,
                                        ],
                                    )
                                if after_first_x is not None:
                                    after_first_x()
                                    after_first_x = None
                                for n in "qkv":
                                    for ks in range(4):
                                        nc.tensor.matmul(
                                            acc[n][:],
                                            w_sb[n][
                                                :, ko * 4 + ks, :
                                            ].bitcast(mmdt),
                                            x_sb[:, ks, :].bitcast(mmdt),
                                            start=(ko == 0 and ks == 0),
                                            stop=(ko == 1 and ks == 3),
                                        )
                            sl = slice(sq * 512, (sq + 1) * 512)
                            nc.scalar.activation(
                                qT_sb[:, sl], acc["q"][:], AF.Identity,
                                bias=bias_sb["q"][:], scale=SCALE,
                            )
                            nc.scalar.activation(
                                kT_sb[:, sl], acc["k"][:], AF.Identity,
                                bias=bias_sb["k"][:],
                            )
                            nc.scalar.activation(
                                vT_sb[:, sl], acc["v"][:], AF.Identity,
                                bias=bias_sb["v"][:],
                            )
                        for hl in range(2):
                            nc.vector.tensor_copy(
                                v_nat[bi * 2 + hl][:, :, 64:65],
                                ones_sb[:, :, None],
                            )

                    def vnat_units(bi, hl):
                        """16 closures: transpose vT columns into v natural
                        [sk, dh] for head (bi, hl). Interleaved into an
                        attention loop so the psum-ring waits hide."""
                        bh = bi * 2 + hl
                        hsl = slice(hl * 64, (hl + 1) * 64)

                        def unit(t):
                            c0 = bi * S + t * P
                            ps = ptile_psum()
                            if vt_dt is f32:
                                pt = ps[:, 0:64]
                            else:
                                pt = ps[:].bitcast(vt_dt)[:, 0:64]
                            nc.tensor.transpose(
                                pt, vT_sb[hsl, c0 : c0 + P], id2[hsl, :]
                            )
                            if t % 2 == 0:
                                nc.vector.tensor_copy(
                                    v_nat[bh][:, t, 0:64], pt
                                )
                            else:
                                nc.scalar.copy(v_nat[bh][:, t, 0:64], pt)

                        return [(lambda t=t: unit(t)) for t in range(16)]

                    _ob_live = {}

                    def norm_units(bh, sqh, ot):
                        """8 closures: transpose back + divide by the
                        denominator row + store o into M (via DRAM scratch,
                        or an SBUF scramble for the last head)."""
                        b, hl = bh // 2, bh % 2

                        def unit(tb):
                            if tb == 0:
                                _ob_live[(bh, sqh)] = obp.tile(
                                    [P, 8, DH], elt, tag="ob", name="ob"
                                )
                            ob = _ob_live[(bh, sqh)]
                            ps = ptile_psum()
                            pt2 = ps[:, 0:72]
                            nc.tensor.transpose(
                                pt2[:, :65],
                                ot[:, tb * P : (tb + 1) * P],
                                id_f32[:65, :65],
                            )
                            rc = rcp.tile([P, 1], f32, tag="rc", name="rc")
                            nc.vector.reciprocal(rc[:], pt2[:, 64:65])
                            if tb % 2 == 0:
                                nc.scalar.activation(
                                    ob[:, tb, :], pt2[:, 0:64], AF.Copy,
                                    scale=rc[:],
                                )
                            else:
                                nc.vector.tensor_scalar_mul(
                                    ob[:, tb, :], pt2[:, 0:64], rc[:]
                                )
                            if tb == 7:
                                s0 = sqh * 1024
                                nc.sync.dma_start(
                                    osc.ap()[
                                        bh, s0 : s0 + 1024, :
                                    ].rearrange("(t p) d -> p t d", p=P),
                                    ob[:],
                                )
                                r0 = sqh * 32
                                nc.sync.dma_start(
                                    M_sb[b][
                                        hl * 64 + r0 : hl * 64 + r0 + 32, :
                                    ],
                                    oscF[
                                        bh, r0 * 2048 : (r0 + 32) * 2048
                                    ].rearrange("(r c) -> r c", c=2048),
                                )

                        return [(lambda tb=tb: unit(tb)) for tb in range(8)]

                    def attention_head(b, hl, extra=None, norm_self=False):
                        """Returns the two ot (o-transposed+denominator)
                        SBUF tiles; caller schedules their normalization.
                        `extra`: closures drained one per half-iteration
                        (64 slots). `norm_self`: normalize in-head (last
                        head)."""
                        bh = b * 2 + hl
                        hsl = slice(hl * 64, (hl + 1) * 64)
                        extra = extra if extra is not None else []
                        ots = []
                        for sqh in range(2):  # halves of 1024 queries
                            sq0 = b * S + sqh * 1024
                            po = ops.tile(
                                [65, 1024], f32, tag="oacc", name="oacc"
                            )

                            def emit_pv(kc, half, pt):
                                nc.tensor.matmul(
                                    po[:, half * 512 : (half + 1) * 512],
                                    v_nat[bh][:, kc, 0:65],
                                    pt[:],
                                    start=(kc == 0),
                                    stop=(kc == 15),
                                )

                            # software-pipelined: pv lags ~2 kc behind qk;
                            # exp alternates ACT (table exp) and DVE
                            # (Schraudolph int-trick, ~3% max rel err; the
                            # denominator uses the same approx values so
                            # common-mode cancels) per 512-query half-tile
                            pending_pv = []
                            for kc in range(16):
                                k0 = b * S + kc * P
                                for half in range(2):
                                    ps = ptile_psum()
                                    nc.tensor.matmul(
                                        ps[:],
                                        kT_sb[hsl, k0 : k0 + P].bitcast(qk_mm),
                                        qT_sb[
                                            hsl,
                                            sq0 + half * 512 : sq0
                                            + (half + 1) * 512,
                                        ].bitcast(qk_mm),
                                        start=True,
                                        stop=True,
                                    )
                                    pt = ptp.tile(
                                        [P, 512], pv_dt, tag="pt", name="pt"
                                    )
                                    if (mode != "f32" and not no_schr
                                            and (kc + half) % 2):
                                        nc.vector.tensor_scalar(
                                            pt[:].bitcast(i16), ps[:],
                                            SCHR_A16, SCHR_B16,
                                            mybir.AluOpType.mult,
                                            mybir.AluOpType.add,
                                        )
                                    else:
                                        nc.scalar.activation(
                                            pt[:], ps[:], AF.Exp
                                        )
                                    pending_pv.append((kc, half, pt))
                                    # deep lag (~4 kc): by the time a pv
                                    # reaches the PE queue head its exp sem
                                    # has long fired, so the in-order queue
                                    # never stalls and LDWEIGHTS prefetches
                                    if len(pending_pv) > pv_lag:
                                        emit_pv(*pending_pv.pop(0))
                                    if extra:
                                        extra.pop(0)()
                            for args in pending_pv:
                                emit_pv(*args)

                            ot = otp.tile([65, 1024], f32, tag="ot", name="ot")
                            # split po evict across both engines: frees the
                            # 2-bank po accumulator sooner for the next half
                            nc.scalar.copy(ot[:, 0:512], po[:, 0:512])
                            nc.vector.tensor_copy(
                                ot[:, 512:1024], po[:, 512:1024]
                            )
                            ots.append(ot)
                            if norm_self and sqh == 0:
                                # queue own first-half normalize into the
                                # second half's slots
                                extra.extend(norm_units(bh, 0, ot))
                        if norm_self:
                            for u in extra:  # leftovers
                                u()
                            for u in norm_units(bh, 1, ots[1]):
                                u()
                        return ots

                    _ysb_live = {}

                    def outproj_halves(b):
                        """32 closures: one 512-col matmul + one psum-evict
                        each; evict engine alternates ACT/DVE."""

                        def unit(mo, nh, half):
                            if nh == 0 and half == 0:
                                _ysb_live[(b, mo)] = ysbp.tile(
                                    [P, 2 * 1024], bf16, tag="y", name="y"
                                )
                            ysb = _ysb_live[(b, mo)]
                            py = ptile_psum()
                            n0 = nh * 1024 + half * 512
                            nc.tensor.matmul(
                                py[:],
                                woT_sb[:, mo * P : (mo + 1) * P].bitcast(mmdt),
                                M_sb[b][:, n0 : n0 + 512].bitcast(mmdt),
                                start=True,
                                stop=True,
                            )
                            if (mo + nh + half) % 2 == 0:
                                nc.vector.tensor_copy(
                                    ysb[:, n0 : n0 + 512], py[:]
                                )
                            else:
                                nc.scalar.copy(ysb[:, n0 : n0 + 512], py[:])
                            if nh == 1 and half == 1:
                                nc.sync.dma_start(
                                    ypT.ap()[b, mo * P : (mo + 1) * P, :],
                                    ysb[:],
                                )

                        return [
                            (lambda mo=mo, nh=nh, half=half: unit(mo, nh, half))
                            for mo in range(8)
                            for nh in range(2)
                            for half in range(2)
                        ]

                    proj_batch(
                        0,
                        after_first_x=(
                            load_rest_of_weights if _rep == 0 else None
                        ),
                    )
                    if _rep == 0:
                        nc.sync.dma_start(woT_sb[:], woT.ap())
                    ot00 = attention_head(
                        0, 0, extra=vnat_units(0, 0) + vnat_units(0, 1)
                    )
                    proj_batch(1)
                    ot01 = attention_head(
                        0, 1,
                        extra=norm_units(0, 0, ot00[0])
                        + norm_units(0, 1, ot00[1])
                        + vnat_units(1, 0),
                    )
                    ot10 = attention_head(
                        1, 0,
                        extra=norm_units(1, 0, ot01[0])
                        + norm_units(1, 1, ot01[1])
                        + vnat_units(1, 1),
                    )
                    op0 = outproj_halves(b=0)
                    attention_head(
                        1, 1,
                        extra=norm_units(2, 0, ot10[0])
                        + norm_units(2, 1, ot10[1])
                        + op0[:16],
                        norm_self=True,
                    )
                    # outproj(0) leftovers fill the window while the last
                    # head's scramble DMAs land in M (emitted after
                    # norm(3,1) so those DMAs are first in the queue);
                    # then the exposed outproj(1)
                    for u in op0[16:]:
                        u()
                    for u in outproj_halves(b=1):
                        u()

    nc.compile()
    return nc


_CACHE = {}


def _np_elt(mode):
    if mode == "bf16":
        import ml_dtypes

        return ml_dtypes.bfloat16
    return np.float32


def _get_runner(mode, reps=1, **build_kw):
    """Build (once) the compiled kernel + a persistent jitted executor."""
    key = (mode, reps, tuple(sorted(build_kw.items())))
    if key in _CACHE:
        return _CACHE[key]

    import jax
    import jax.numpy as jnp  # noqa: F401
    from jax.sharding import Mesh, PartitionSpec
    from jax.experimental.shard_map import shard_map
    import concourse.mybir as mybir
    from concourse import bass2jax

    nc = _build_nc(mode, reps, **build_kw)
    bass2jax.install_neuronx_cc_hook()

    partition_name = (
        nc.partition_id_tensor.name if nc.partition_id_tensor else None
    )
    in_names = []
    out_names = []
    out_avals = []
    for alloc in nc.m.functions[0].allocations:
        if not isinstance(alloc, mybir.MemoryLocationSet):
            continue
        name = alloc.memorylocations[0].name
        if alloc.kind == "ExternalInput":
            if name != partition_name:
                in_names.append(name)
        elif alloc.kind == "ExternalOutput":
            out_names.append(name)
            shape = tuple(alloc.tensor_shape)
            dtype = mybir.dt.np(alloc.dtype)
            out_avals.append(jax.core.ShapedArray(shape, dtype))
    n_params = len(in_names)
    n_outs = len(out_avals)
    all_in_names = list(in_names) + list(out_names)
    if partition_name is not None:
        all_in_names.append(partition_name)
    all_in_names = tuple(all_in_names)

    def _body(*args):
        operands = list(args)
        if partition_name is not None:
            operands.append(bass2jax.partition_id_tensor())
        outs = bass2jax._bass_exec_p.bind(
            *operands,
            out_avals=tuple(out_avals),
            in_names=all_in_names,
            out_names=tuple(out_names),
            lowering_input_output_aliases=(),
            sim_require_finite=True,
            sim_require_nnan=True,
            nc=nc,
        )
        return tuple(outs)

    devices = jax.devices()[:N_CORES]
    mesh = Mesh(np.asarray(devices), ("core",))
    in_specs = (PartitionSpec("core"),) * (n_params + n_outs)
    out_specs = (PartitionSpec("core"),) * n_outs
    donate = tuple(range(n_params, n_params + n_outs))
    sharded = jax.jit(
        shard_map(
            _body, mesh=mesh, in_specs=in_specs, out_specs=out_specs,
            check_rep=False,
        ),
        donate_argnums=donate,
        keep_unused=True,
    )

    zero_out_shapes = [
        ((N_CORES * a.shape[0],) + tuple(a.shape[1:]), a.dtype)
        for a in out_avals
    ]

    def execute(in_maps):
        concat_in = [
            np.concatenate([np.asarray(m[name]) for m in in_maps], axis=0)
            for name in in_names
        ]
        concat_zeros = [np.zeros(s, d) for s, d in zero_out_shapes]
        out_arrs = sharded(*concat_in, *concat_zeros)
        out_arrs = [np.asarray(o) for o in out_arrs]
        return [
            {
                name: out_arrs[i].reshape(
                    N_CORES, *out_avals[i].shape
                )[c]
                for i, name in enumerate(out_names)
            }
            for c in range(N_CORES)
        ]

    execute.nc = nc
    execute.in_names = in_names
    execute.out_names = out_names
    execute.out_avals = out_avals
    execute.n_params = n_params
    execute.body = _body
    execute.mesh = mesh
    execute.zero_out_shapes = zero_out_shapes
    _CACHE[key] = execute
    return execute


def make_in_maps(x, Wq, bq, Wk, bk, Wv, bv, Wo, bo, mode=None):
    mode = mode or DTYPE_MODE
    ne = _np_elt(mode)
    x = np.asarray(x, np.float32)
    xT = np.ascontiguousarray(x.reshape(NSEQ, D).T).astype(ne)
    in_maps = []
    for c in range(N_CORES):
        sl = slice(c * P, (c + 1) * P)
        in_maps.append(
            {
                "xT": xT,
                "wqT": np.ascontiguousarray(np.asarray(Wq)[sl, :].T).astype(ne),
                "wkT": np.ascontiguousarray(np.asarray(Wk)[sl, :].T).astype(ne),
                "wvT": np.ascontiguousarray(np.asarray(Wv)[sl, :].T).astype(ne),
                "woT": np.ascontiguousarray(np.asarray(Wo)[:, sl].T).astype(ne),
                "bqs": (np.asarray(bq, np.float32)[sl] * SCALE).reshape(P, 1),
                "bk": np.asarray(bk, np.float32)[sl].reshape(P, 1).copy(),
                "bv": np.asarray(bv, np.float32)[sl].reshape(P, 1).copy(),
            }
        )
    return in_maps


def kernel(x, Wq, bq, Wk, bk, Wv, bv, Wo, bo):
    mode = DTYPE_MODE
    execute = _get_runner(mode)
    in_maps = make_in_maps(x, Wq, bq, Wk, bk, Wv, bv, Wo, bo, mode)
    results = execute(in_maps)
    ysum = np.zeros((B, D, S), np.float64)
    for c in range(N_CORES):
        ysum += np.asarray(results[c]["ypT"], np.float32)
    y = ysum.transpose(0, 2, 1) + np.asarray(bo, np.float32)[None, None, :]
    return np.ascontiguousarray(y.astype(np.float32))

